# revision 15
# baseline (speedup 1.0000x reference)
"""Chunk-parallel gated delta rule kernel for TRN2 (8 NeuronCores).

Algorithm (per (b,h) scan, chunk size C=128):
  Within a chunk, the delta-rule recurrence
      S_t = exp(g_t) S_{t-1} + k_t u_t^T,  u_t = b_t (v_t - k_t^T exp(g_t) S_{t-1})
  is solved in closed form via the WY / UT transform:
      U = (I + W)^{-1} (bV - bA.K @ S_0),   W[t,s] = b_t (A_t/A_s) k_t.k_s  (s<t)
      O = (A.Q) @ S_0 + tril((A_t/A_s) q_t.k_s) @ U
      S_1 = A_C S_0 + ((A_C/A_t) k_t)^T @ U
  with A_t = exp(cumsum g).  The triangular inverse is computed by nilpotent
  doubling: (I - X)^{-1} = prod_j (I + X^(2^j)), X = -W strictly lower
  (truncated at X^8: dropped terms are ~1e-5 of the smallest X entries and
  measure 7e-12 end-to-end against the exact inverse on reference data).

Sharding: B*H = 64 independent scans -> 8 per core, interleaved so the serial
chunk chain of one head hides under the parallel work of the others.

Engine budget per (chunk, head) iteration: 20 matmuls on PE, ~13 ops on DVE
(copies/masked moves/adds), ~9 cheap scalings on GpSimd, ~nothing on ACT
(activations cost ~1.7us each on trn2 -- the Sqrt is batched once per chunk).
"""

import numpy as np

import concourse.bass as bass
import concourse.mybir as mybir
from concourse import bacc
from concourse.bass import MemorySpace
from concourse.bass_utils import run_bass_kernel_spmd
from concourse.masks import make_identity, make_lower_triangular, make_upper_triangular
from concourse.tile import TileContext

B, H, T, K, V = 4, 16, 2048, 128, 128
NCORES = 8
NBH = (B * H) // NCORES  # 8 scans per core
C = 128                  # chunk size
NCH = T // C             # 16 chunks
ND = 3                   # doubling steps (see module docstring)
F32 = mybir.dt.float32
AX = mybir.AluOpType
ACT = mybir.ActivationFunctionType


def build_nc(n_bh=NBH, nch=NCH, nd=ND):
    Tt = nch * C
    nc = bacc.Bacc(None, target_bir_lowering=False)
    q_d = nc.declare_dram_parameter("q", [n_bh, Tt, K], F32, isOutput=False)
    k_d = nc.declare_dram_parameter("k", [n_bh, Tt, K], F32, isOutput=False)
    v_d = nc.declare_dram_parameter("v", [n_bh, Tt, V], F32, isOutput=False)
    g_d = nc.declare_dram_parameter("g", [n_bh, Tt], F32, isOutput=False)
    b_d = nc.declare_dram_parameter("beta", [n_bh, Tt], F32, isOutput=False)
    s0_d = nc.declare_dram_parameter("s0", [n_bh, K, V], F32, isOutput=False)
    o_d = nc.declare_dram_parameter("o", [n_bh, Tt, V], F32, isOutput=True)
    sf_d = nc.declare_dram_parameter("sf", [n_bh, K, V], F32, isOutput=True)

    with TileContext(nc) as tc:
        with (
            tc.tile_pool(name="consts", bufs=1) as cpool,
            tc.tile_pool(name="state", bufs=1) as spool,
            tc.tile_pool(name="work", bufs=4) as wpool,
            tc.tile_pool(name="deep", bufs=3) as dpool,
            tc.tile_pool(name="psum", bufs=8, space=MemorySpace.PSUM) as ppool,
        ):
            # ---- constants ----
            ident = cpool.tile([128, 128], F32, tag="ident")
            make_identity(nc, ident)
            ut_incl = cpool.tile([128, 128], F32, tag="ut_incl")  # 1 if p<=f
            make_upper_triangular(nc, ut_incl, val=1.0, diag=True)
            ones = cpool.tile([128, 128], F32, tag="ones")
            nc.gpsimd.memset(ones, 1.0)
            # [strict upper | upper incl | strict lower] for the [XT|PT|X] move
            mask3 = cpool.tile([128, 384], F32, tag="mask3")
            make_upper_triangular(nc, mask3[:, 0:128], val=1.0, diag=False)
            nc.vector.tensor_copy(mask3[:, 128:256], ut_incl)
            make_lower_triangular(nc, mask3[:, 256:384], val=1.0, diag=False)
            lnq = cpool.tile([128, 1], F32, tag="lnq")
            nc.gpsimd.memset(lnq, float(-0.5 * np.log(K)))

            # Warmup: make PE observe the gpsimd const-building tick before the
            # first real transpose (transposes can encode only one sync wait).
            warm_ps = ppool.tile([16, 16], F32, tag="ps")
            nc.tensor.transpose(warm_ps, ident[:16, :16], ident[:16, :16])
            warm_sb = cpool.tile([16, 16], F32, tag="warm_sb")
            nc.vector.tensor_copy(warm_sb, warm_ps)

            # ---- persistent tiles ----
            S_sb = [
                spool.tile([K, V], F32, tag=f"S{i}", name=f"S{i}")
                for i in range(n_bh)
            ]
            W = n_bh * nch  # one column per (scan, chunk)
            Aq8 = spool.tile([C, W], F32, tag="Aq8")      # exp(gcum)/sqrt(K)
            Ainv8 = spool.tile([C, W], F32, tag="Ainv8")  # exp(-gcum)
            beT8 = spool.tile([C, W], F32, tag="beT8")    # beta (time-major)
            nbA8 = spool.tile([C, W], F32, tag="nbA8")    # -beta*exp(gcum)
            stl8 = spool.tile([C, W], F32, tag="stl8")    # exp(g_tot-gcum)
            ACc8 = spool.tile([C, W], F32, tag="ACc8")    # exp(g_tot) bcast

            # ---- batched preprocessing (all scans at once) ----
            gT_ps = ppool.tile([C, W], F32, tag="ps")
            bT_ps = ppool.tile([C, W], F32, tag="ps")
            for i in range(n_bh):
                nc.sync.dma_start(out=S_sb[i], in_=s0_d[i])
                gb = wpool.tile([nch, C], F32, tag="gb")
                nc.sync.dma_start(out=gb, in_=g_d[i].rearrange("(n c) -> n c", c=C))
                bb = wpool.tile([nch, C], F32, tag="bb")
                nc.sync.dma_start(out=bb, in_=b_d[i].rearrange("(n c) -> n c", c=C))
                nc.tensor.transpose(
                    gT_ps[:, i * nch : (i + 1) * nch], gb, ident[:nch, :nch]
                )
                nc.tensor.transpose(
                    bT_ps[:, i * nch : (i + 1) * nch], bb, ident[:nch, :nch]
                )
            gT8 = wpool.tile([C, W], F32, tag="gT8")
            nc.vector.tensor_copy(gT8, gT_ps)
            nc.vector.tensor_copy(beT8, bT_ps)

            gcum_ps = ppool.tile([C, W], F32, tag="ps")
            nc.tensor.matmul(gcum_ps, ut_incl, gT8, start=True, stop=True)
            glast_ps = ppool.tile([C, W], F32, tag="ps")
            nc.tensor.matmul(glast_ps, ones, gT8, start=True, stop=True)

            nc.scalar.activation(Aq8, gcum_ps, ACT.Exp, bias=lnq[:, 0:1])
            nc.scalar.activation(Ainv8, gcum_ps, ACT.Exp, scale=-1.0)
            nc.scalar.activation(ACc8, glast_ps, ACT.Exp)
            nc.vector.tensor_tensor(nbA8, beT8, Aq8, op=AX.mult)
            nc.gpsimd.tensor_scalar(nbA8, nbA8, float(-np.sqrt(K)), None, AX.mult)
            nc.vector.tensor_tensor(stl8, ACc8, Ainv8, op=AX.mult)

            # ---- main loop: phase-sliced so the 8 scans pipeline ----
            # Phase 0 (loads + norms) of chunk c+1 is emitted before the main
            # phases of chunk c, hiding the per-chunk sqrt barrier.
            R = range(n_bh)

            def phase0(c):
                tsl = slice(c * C, (c + 1) * C)
                ssq = wpool.tile([C, 2 * n_bh], F32, tag="ssq", name="ssq")
                qkv = []
                for i in R:
                    q_c = wpool.tile([C, K], F32, tag=f"q{i}", bufs=2, name=f"q{i}")
                    nc.sync.dma_start(out=q_c, in_=q_d[i, tsl, :])
                    k_c = wpool.tile([C, K], F32, tag=f"k{i}", bufs=2, name=f"k{i}")
                    nc.sync.dma_start(out=k_c, in_=k_d[i, tsl, :])
                    v_c = wpool.tile([C, V], F32, tag=f"v{i}", bufs=2, name=f"v{i}")
                    nc.sync.dma_start(out=v_c, in_=v_d[i, tsl, :])
                    qkv.append((q_c, k_c, v_c))
                for i in R:
                    q_c, k_c, _ = qkv[i]
                    sq = wpool.tile([C, 2 * K], F32, tag="sq", bufs=6, name="sq")
                    nc.gpsimd.tensor_tensor(sq[:, 0:K], q_c, q_c, op=AX.mult)
                    nc.gpsimd.tensor_tensor(sq[:, K : 2 * K], k_c, k_c, op=AX.mult)
                    nc.vector.reduce_sum(
                        ssq[:, 2 * i : 2 * i + 2],
                        sq.rearrange("p (a b) -> p a b", a=2),
                        axis=mybir.AxisListType.X,
                    )
                rn = wpool.tile([C, 2 * n_bh], F32, tag="rn", name="rn")
                nc.scalar.activation(rn, ssq, ACT.Sqrt)
                rec = wpool.tile([C, 2 * n_bh], F32, tag="rec", name="rec")
                nc.vector.reciprocal(rec, rn)
                return qkv, rec

            pending = phase0(0)
            for c in range(nch):
                tsl = slice(c * C, (c + 1) * C)
                cols = [i * nch + c for i in R]
                qkv, rec = pending

                # phase 1: scaled operand tiles (GpSimd per-row scalings)
                QA_l, K2_l, KnbA_l, Ktil_l, bV_l = [], [], [], [], []
                for i in R:
                    q_c, k_c, v_c = qkv[i]
                    col = cols[i]
                    qsc = wpool.tile([C, 1], F32, tag="qsc", bufs=6)
                    nc.vector.tensor_tensor(
                        qsc, Aq8[:, col : col + 1], rec[:, 2 * i : 2 * i + 1],
                        op=AX.mult,
                    )
                    QA = wpool.tile([C, K], F32, tag=f"QA{i}", bufs=2, name=f"QA{i}")
                    nc.gpsimd.tensor_scalar(QA, q_c, qsc[:, 0:1], None, AX.mult)
                    kn = wpool.tile([C, K], F32, tag="kn", bufs=4)
                    nc.gpsimd.tensor_scalar(
                        kn, k_c, rec[:, 2 * i + 1 : 2 * i + 2], None, AX.mult
                    )
                    K2 = wpool.tile([C, K], F32, tag=f"K2{i}", bufs=2, name=f"K2{i}")
                    nc.gpsimd.tensor_scalar(
                        K2, kn, Ainv8[:, col : col + 1], None, AX.mult
                    )
                    KnbA = wpool.tile(
                        [C, K], F32, tag=f"KnbA{i}", bufs=2, name=f"KnbA{i}"
                    )
                    nc.gpsimd.tensor_scalar(
                        KnbA, kn, nbA8[:, col : col + 1], None, AX.mult
                    )
                    Ktil = wpool.tile(
                        [C, K], F32, tag=f"Ktil{i}", bufs=2, name=f"Ktil{i}"
                    )
                    nc.gpsimd.tensor_scalar(
                        Ktil, kn, stl8[:, col : col + 1], None, AX.mult
                    )
                    bV = wpool.tile([C, V], F32, tag=f"bV{i}", bufs=2, name=f"bV{i}")
                    nc.gpsimd.tensor_scalar(
                        bV, v_c, beT8[:, col : col + 1], None, AX.mult
                    )
                    QA_l.append(QA); K2_l.append(K2)
                    KnbA_l.append(KnbA); Ktil_l.append(Ktil); bV_l.append(bV)

                # prefetch next chunk's loads + norms behind this chunk's work
                if c + 1 < nch:
                    pending = phase0(c + 1)

                # phase 2: feature-major transposes [K2T | KnbAT | QAT],
                # one PSUM bank per scan; PSUM->SBUF move on the idle ACT
                tri_l = []
                for i in R:
                    tri_ps = ppool.tile([128, 384], F32, tag="ps")
                    nc.tensor.transpose(tri_ps[:, 0:128], K2_l[i], ident)
                    nc.tensor.transpose(tri_ps[:, 128:256], KnbA_l[i], ident)
                    nc.tensor.transpose(tri_ps[:, 256:384], QA_l[i], ident)
                    tri = wpool.tile([128, 384], F32, tag=f"tri{i}", bufs=2,
                                     name=f"tri{i}")
                    nc.scalar.activation(tri, tri_ps, ACT.Copy)
                    tri_l.append(tri)

                # phase 3: [XT | PT] in one N=256 matmul (shared lhsT=K2T),
                # X separately; one masked move for all three
                xpx_l, prod_l = [], []
                for i in R:
                    K2T, KnbAT, QAT = (
                        tri_l[i][:, 0:128], tri_l[i][:, 128:256], tri_l[i][:, 256:384]
                    )
                    xpx_ps = ppool.tile([128, 384], F32, tag="ps")
                    nc.tensor.matmul(
                        xpx_ps[:, 0:256], K2T, tri_l[i][:, 128:384],
                        start=True, stop=True,
                    )
                    nc.tensor.matmul(
                        xpx_ps[:, 256:384], KnbAT, K2T, start=True, stop=True
                    )
                    xpx = wpool.tile([128, 384], F32, tag=f"xpx{i}", bufs=2,
                                     name=f"xpx{i}")
                    nc.vector.tensor_tensor(xpx, xpx_ps, mask3, op=AX.mult)
                    xpx_l.append(xpx)
                    prod = dpool.tile([128, 128], F32, tag=f"prod{i}", bufs=2,
                                      name=f"prod{i}")
                    nc.gpsimd.tensor_tensor(prod, xpx[:, 0:128], ident, op=AX.add)
                    prod_l.append(prod)

                # phase 4: nilpotent doubling, stepped across all scans.
                # Producer sweep (all squarings) before consumer sweep (all
                # product updates) so PE never stalls on a just-written copy.
                Xj_l = [x[:, 256:384] for x in xpx_l]
                XTj_l = [x[:, 0:128] for x in xpx_l]
                for j in range(nd):
                    last = j == nd - 1
                    w = 128 if last else 256
                    xx_l = []
                    for i in R:
                        sq_ps = ppool.tile([128, w], F32, tag="ps")
                        nc.tensor.matmul(
                            sq_ps[:, 0:128], XTj_l[i], Xj_l[i], start=True, stop=True
                        )
                        if not last:
                            nc.tensor.matmul(
                                sq_ps[:, 128:256], Xj_l[i], XTj_l[i],
                                start=True, stop=True,
                            )
                        xx = dpool.tile([128, w], F32, tag=f"xx{i}", bufs=2,
                                        name=f"xx{i}")
                        nc.vector.tensor_copy(xx, sq_ps)
                        xx_l.append(xx)
                    for i in R:
                        pr_ps = ppool.tile([128, 128], F32, tag="ps")
                        nc.tensor.matmul(
                            pr_ps, xx_l[i][:, 0:128], prod_l[i], start=True, stop=True
                        )
                        prod_n = dpool.tile([128, 128], F32, tag=f"prod{i}", bufs=2,
                                            name=f"prod{i}n")
                        nc.vector.tensor_tensor(prod_n, pr_ps, prod_l[i], op=AX.add)
                        prod_l[i] = prod_n
                        Xj_l[i] = xx_l[i][:, 0:128]
                        if not last:
                            XTj_l[i] = xx_l[i][:, 128:256]

                # phase 5: WnT = KnbA' @ TT (mm sweep, then copy sweep)
                WnT_l = []
                for i in R:
                    wnt_ps = ppool.tile([K, C], F32, tag="ps")
                    nc.tensor.matmul(wnt_ps, KnbA_l[i], prod_l[i], start=True, stop=True)
                    WnT = wpool.tile([K, C], F32, tag=f"WnT{i}", bufs=2,
                                     name=f"WnT{i}")
                    nc.vector.tensor_copy(WnT, wnt_ps)
                    WnT_l.append(WnT)

                # phase 6: U = T @ bV - (T @ bA.K) @ S0  (serial link via S)
                u_ps_l = []
                for i in R:
                    u_ps = ppool.tile([C, V], F32, tag="ps")
                    nc.tensor.matmul(u_ps, WnT_l[i], S_sb[i], start=True, stop=False)
                    nc.tensor.matmul(u_ps, prod_l[i], bV_l[i], start=False, stop=True)
                    u_ps_l.append(u_ps)
                U_l = []
                for i in R:
                    U = wpool.tile([C, V], F32, tag=f"U{i}", bufs=2, name=f"U{i}")
                    nc.vector.tensor_copy(U, u_ps_l[i])
                    U_l.append(U)

                # phase 7: outputs + state update (mm sweeps before copy sweeps)
                o_ps_l, s_ps_l = [], []
                for i in R:
                    QAT, PT = tri_l[i][:, 256:384], xpx_l[i][:, 128:256]
                    o_ps = ppool.tile([C, V], F32, tag="ps")
                    nc.tensor.matmul(o_ps, QAT, S_sb[i], start=True, stop=False)
                    nc.tensor.matmul(o_ps, PT, U_l[i], start=False, stop=True)
                    o_ps_l.append(o_ps)
                    s_ps = ppool.tile([K, V], F32, tag="ps")
                    nc.tensor.matmul(s_ps, Ktil_l[i], U_l[i], start=True, stop=True)
                    s_ps_l.append(s_ps)
                for i in R:
                    o_sb = wpool.tile([C, V], F32, tag=f"o_sb{i}", bufs=2,
                                      name=f"o_sb{i}")
                    nc.vector.tensor_copy(o_sb, o_ps_l[i])
                    nc.sync.dma_start(out=o_d[i, tsl, :], in_=o_sb)
                    sdec = wpool.tile([K, V], F32, tag=f"sdec{i}", bufs=2,
                                      name=f"sdec{i}")
                    nc.gpsimd.tensor_scalar(
                        sdec, S_sb[i], ACc8[:, cols[i] : cols[i] + 1], None, AX.mult
                    )
                    nc.vector.tensor_tensor(S_sb[i], sdec, s_ps_l[i], op=AX.add)

            for i in range(n_bh):
                nc.sync.dma_start(out=sf_d[i], in_=S_sb[i])

    nc.compile()
    nc.finalize()
    return nc


_NC_CACHE = {}


def _get_nc(key=(NBH, NCH, ND)):
    if key not in _NC_CACHE:
        _NC_CACHE[key] = build_nc(*key)
    return _NC_CACHE[key]


def kernel(q, k, v, g, beta, initial_state):
    bh = B * H
    qf = np.ascontiguousarray(np.asarray(q, np.float32).reshape(bh, T, K))
    kf = np.ascontiguousarray(np.asarray(k, np.float32).reshape(bh, T, K))
    vf = np.ascontiguousarray(np.asarray(v, np.float32).reshape(bh, T, V))
    gf = np.ascontiguousarray(np.asarray(g, np.float32).reshape(bh, T))
    bf = np.ascontiguousarray(np.asarray(beta, np.float32).reshape(bh, T))
    sf = np.ascontiguousarray(np.asarray(initial_state, np.float32).reshape(bh, K, V))

    nc = _get_nc()
    in_maps = []
    for cid in range(NCORES):
        sl = slice(cid * NBH, (cid + 1) * NBH)
        in_maps.append(
            {"q": qf[sl], "k": kf[sl], "v": vf[sl], "g": gf[sl],
             "beta": bf[sl], "s0": sf[sl]}
        )
    res = run_bass_kernel_spmd(nc, in_maps, list(range(NCORES))).results
    o = np.concatenate([r["o"] for r in res], axis=0).reshape(B, H, T, V)
    s_f = np.concatenate([r["sf"] for r in res], axis=0).reshape(B, H, K, V)
    return o, s_f
